# revision 35
# baseline (speedup 1.0000x reference)
"""Distributed 2-layer GCN on 8 Trainium2 NeuronCores (Bass/Tile).

Strategy (node partition over 8 cores, host-mediated halo exchange):
  Launch A: per-core T = x_shard @ W1               (dense f16 matmul)
  host:     allgather T shards -> T_full
  host:     expand per-edge payload  g[e] = T[src(e)] * norm(e) * 16  (fp8)
            into a degree-sorted, slot-aligned layout: dst node = SBUF
            partition (slot), k-th incident edge = k-th chunk column.
  Launch B: per-core aggregation = PSUM accumulation of payload chunks
            via fp8 DoubleRow matmuls with a constant identity lhsT
            (2 chunks per instruction), then h = relu(agg/16),
            transpose (TensorE) and Z^T = W2^T @ h^T.
  host:     allgather Z shards, expand z[e] = Z[src(e)] * norm(e) * 64 (fp8)
  Launch C: same identity-accumulate aggregation, out = agg (f16);
            host applies /64 and + b2.

No dma_gather / GPSIMD anywhere: the gather indices are known on the host
between launches, so all device traffic is large contiguous DMA.  The
one-hot scatter matrices of the old design are gone too - the slot-aligned
layout makes the aggregation a pure chunk sum, which the identity matmul
performs in PSUM at 2 chunks/instruction (fp8 DoubleRow).
All normalization (D^-1/2 (A+I) D^-1/2) is folded into the payload on the
host at f32/f16 precision with a single fp8 quantization per layer.
b1/b2: b1 is added into the self-loop payload rows (exact when b1=0), b2 is
added on the host after the final gather.
"""

import os
import sys
import types

import ml_dtypes
import numpy as np

import concourse.bass as bass
import concourse.bacc as bacc
import concourse.tile as tile
from concourse import mybir
from concourse.bass_utils import run_bass_kernel_spmd

NCORES = 8
N = 50000
FIN = 768
HID = 256
FOUT = 128
NLOC = N // NCORES            # 6250 nodes per core
NBLK = 49                     # dst blocks per core (49*128 = 6272 slots)
P = 128
NLOC_PAD = NBLK * P

SCALE_B = 16.0                # payload scale for layer-1 messages (fp8 range)
SCALE_C = 64.0                # payload scale for layer-2 messages

F16 = mybir.dt.float16
F32 = mybir.dt.float32
F8 = mybir.dt.float8e4
DRMODE = mybir.MatmulPerfMode.DoubleRow
F8NP = ml_dtypes.float8_e4m3fn

_KC = FIN // P  # 6


def _ensure_ntff_hook():
    """Provide antenv.axon_hooks if the image lacks it, so trace=True works."""
    try:
        import antenv.axon_hooks  # noqa: F401
        return
    except ImportError:
        pass
    import antenv
    mod = types.ModuleType("antenv.axon_hooks")
    mod._hook = None

    def set_axon_ntff_profile_hook(hook):
        mod._hook = hook

    def get_axon_ntff_profile_hook():
        return mod._hook

    mod.set_axon_ntff_profile_hook = set_axon_ntff_profile_hook
    mod.get_axon_ntff_profile_hook = get_axon_ntff_profile_hook
    sys.modules["antenv.axon_hooks"] = mod
    antenv.axon_hooks = mod
    try:
        from trn_agent_boot.trn_boot import _ntff_profile_via_ctypes
        hook = _ntff_profile_via_ctypes("/opt/axon/libaxon_pjrt.so")
        if hook is not None:
            mod._hook = hook
    except Exception:
        pass


def _preprocess(edge_index):
    """Degree-sorted node->(block, slot) assignment per core plus the
    (slot, chunk) placement of every edge (self-loops at chunk 0)."""
    src = edge_index[0].astype(np.int64)
    dst = edge_index[1].astype(np.int64)
    deg = np.bincount(dst, minlength=N).astype(np.float64) + 1.0  # incl self
    dinv = 1.0 / np.sqrt(deg)

    perms = []
    prof = np.zeros(NBLK, np.int64)
    for c in range(NCORES):
        lo = c * NLOC
        dloc = deg[lo:lo + NLOC].astype(np.int64)
        order = np.argsort(-dloc, kind="stable")
        perm_slots = np.empty(NLOC, np.int64)
        perm_slots[order] = np.arange(NLOC)     # node -> b*128 + slot
        dpad = np.zeros(NLOC_PAD, np.int64)
        dpad[:NLOC] = dloc[order]
        cpb = dpad.reshape(NBLK, P).max(axis=1)
        cpb = ((cpb + 1) // 2) * 2              # even for DoubleRow pairing
        prof = np.maximum(prof, cpb)
        perms.append(perm_slots)

    cp = np.maximum(prof, 8)                     # aligned chunk profile
                                                 # (>=8 so C's oct loop runs)
    coff = np.concatenate([[0], np.cumsum(cp)])[:-1].astype(np.int64)
    ct = int(cp.sum())

    pre = []
    for c in range(NCORES):
        lo = c * NLOC
        perm_slots = perms[c]
        sel = (dst >= lo) & (dst < lo + NLOC)
        s_c = src[sel]
        d_glob = dst[sel]
        d_c = d_glob - lo
        n_c = (dinv[s_c] * dinv[d_glob]).astype(np.float32)
        o = np.argsort(d_c, kind="stable")
        s_c, d_c, n_c = s_c[o], d_c[o], n_c[o]
        cnt = np.bincount(d_c, minlength=NLOC)
        starts = np.zeros(NLOC, np.int64)
        starts[1:] = np.cumsum(cnt)[:-1]
        kpos = np.arange(len(d_c)) - starts[d_c] + 1   # 1.. (0 = self)
        pos = perm_slots[d_c]
        blk, slot = pos // P, pos % P
        col = coff[blk] + kpos

        srcmat = np.zeros((P, ct), np.int64)
        normmat = np.zeros((P, ct), np.float32)
        srcmat[slot, col] = s_c
        normmat[slot, col] = n_c
        # self loops at chunk 0 of each block
        nodes = np.arange(NLOC)
        posn = perm_slots[nodes]
        blkn, slotn = posn // P, posn % P
        srcmat[slotn, coff[blkn]] = lo + nodes
        normmat[slotn, coff[blkn]] = (dinv[lo + nodes] ** 2).astype(np.float32)
        pre.append({"perm": posn, "srcmat": srcmat, "normmat": normmat})
    return pre, cp, coff, ct, dinv


def _build_a():
    nc = bacc.Bacc("TRN2", target_bir_lowering=False, debug=False,
                   num_devices=NCORES)
    # host-swizzled so each block loads as one contiguous-per-partition DMA:
    # xtb[b, p, k*128+n] = x[b*128+n, k*128+p]
    t_xt = nc.dram_tensor("xtb", [NBLK, P, FIN], F16, kind="ExternalInput")
    t_w1 = nc.dram_tensor("w1", [FIN, HID], F16, kind="ExternalInput")
    t_out = nc.dram_tensor("t_out", [NLOC_PAD, HID], F16, kind="ExternalOutput")
    with tile.TileContext(nc) as tc:
        with (
            tc.tile_pool(name="const", bufs=1) as cs,
            tc.tile_pool(name="sb", bufs=8) as sb,
            tc.tile_pool(name="tp", bufs=1) as tp,
            tc.tile_pool(name="ps", bufs=4, space="PSUM") as ps,
        ):
            w1t = cs.tile([P, _KC * HID], F16)
            for k in range(_KC):
                nc.sync.dma_start(w1t[:, k * HID:(k + 1) * HID],
                                  t_w1[k * P:(k + 1) * P, :])

            def _epilogue_a(b, pt):
                ts = tp.tile([P, HID], F16, tag=f"ts{b}", name=f"ts{b}")
                nc.scalar.activation(out=ts[:], in_=pt[:],
                                     func=mybir.ActivationFunctionType.Copy,
                                     bias=0.0, scale=1.0)
                nc.sync.dma_start(t_out[b * P:(b + 1) * P, :], ts[:])

            prev = None
            for b in range(NBLK):
                xts = sb.tile([P, FIN], F16, tag="xt")
                nc.sync.dma_start(xts[:], t_xt[b])
                pt = ps.tile([P, HID], F32, tag="pt")
                for k in range(_KC):
                    nc.tensor.matmul(pt[:], lhsT=xts[:, k * P:(k + 1) * P],
                                     rhs=w1t[:, k * HID:(k + 1) * HID],
                                     start=(k == 0), stop=(k == _KC - 1))
                if prev is not None:
                    _epilogue_a(*prev)
                prev = (b, pt)
            _epilogue_a(*prev)
    nc.compile()
    return nc


def _build_agg(cp, is_b):
    """Aggregation launch: identity-accumulate over slot-aligned payload.
    B (elem=HID): h = relu(agg/16), transpose, Z^T = W2^T h^T.
    C (elem=FOUT): out = agg (f16)."""
    ct = int(np.sum(cp))
    cpmax = int(np.max(cp))
    elem = HID if is_b else FOUT
    nc = bacc.Bacc("TRN2", target_bir_lowering=False, debug=False,
                   num_devices=NCORES)
    t_g = nc.dram_tensor("gexp", [P, ct * elem], F8, kind="ExternalInput")
    t_ip = nc.dram_tensor("ipair", [P, 2 * P], F8, kind="ExternalInput")
    if is_b:
        t_i16 = nc.dram_tensor("i16", [P, P], F16, kind="ExternalInput")
        t_w2 = nc.dram_tensor("w2", [HID, FOUT], F16, kind="ExternalInput")
        t_o = nc.dram_tensor("zt_out", [FOUT, NLOC_PAD], F16,
                             kind="ExternalOutput")
    else:
        t_o = nc.dram_tensor("ot_out", [NLOC_PAD, FOUT], F16,
                             kind="ExternalOutput")

    with tile.TileContext(nc) as tc:
        with (
            tc.tile_pool(name="const", bufs=1) as cs,
            tc.tile_pool(name="gp", bufs=10 if is_b else 4) as gp,
            tc.tile_pool(name="hp", bufs=1) as hp,
            tc.tile_pool(name="sb", bufs=8) as sb,
            tc.tile_pool(name="ps", bufs=3 if is_b else 8, space="PSUM") as ps,
            tc.tile_pool(name="ps2", bufs=2, space="PSUM") as ps2,
            tc.tile_pool(name="ps3", bufs=3, space="PSUM") as ps3,
        ):
            ip = cs.tile([P, 2 * P], F8)
            nc.sync.dma_start(ip[:], t_ip[:, :])
            ip3 = ip[:].rearrange("p (two f) -> p two f", two=2)
            if is_b:
                i16 = cs.tile([P, P], F16)
                nc.sync.dma_start(i16[:], t_i16[:, :])
                w2t = cs.tile([P, 2 * FOUT], F16)
                for k in range(2):
                    nc.sync.dma_start(w2t[:, k * FOUT:(k + 1) * FOUT],
                                      t_w2[k * P:(k + 1) * P, :])

            # phase 1: payload DMA + identity-DR aggregation, chain after
            # chain. Each quad-DR matmul contracts 4 chunks into two
            # side-by-side half-aggregates; DVE adds the halves. h lands in
            # a distinct SBUF tile per block (no ring WAR), so TensorE
            # streams without cross-engine stalls.
            hs = []
            # C: group 4 blocks per payload DMA (fewer, larger transfers);
            # B: per-block DMA (bus-bound anyway, SBUF is tighter there).
            grp = 1 if is_b else 4
            goffs = {}
            gtiles = {}
            off = 0
            for b in range(NBLK):
                nch = int(cp[b])
                if b % grp == 0:
                    gn = sum(int(cp[i]) for i in range(b, min(b + grp, NBLK)))
                    gt = gp.tile([P, grp * cpmax * elem], F8, tag="g")
                    nc.sync.dma_start(gt[:, 0:gn * elem],
                                      t_g[:, off * elem:(off + gn) * elem])
                    gtiles[b] = gt
                    goffs[b] = 0
                else:
                    gtiles[b] = gtiles[b - 1]
                    goffs[b] = goffs[b - 1] + int(cp[b - 1])
                gt = gtiles[b]
                gofs = goffs[b] * elem
                # per-instruction chunk span: B quads (free 2x512), C octs
                # (free 2x512 = 8 chunks of 128) -> PSUM [128, W] partials
                span = 4 if is_b else 8
                agg = ps.tile([P, 512], F32, tag="agg")
                nfull, rem = nch // span, nch % span
                for j in range(nfull):
                    rhs = gt[:, gofs + span * j * elem:
                             gofs + span * (j + 1) * elem].rearrange(
                        "p (two f) -> p two f", two=2)
                    nc.tensor.matmul(agg[:], lhsT=ip3, rhs=rhs,
                                     start=(j == 0),
                                     stop=(rem == 0 and j == nfull - 1),
                                     perf_mode=DRMODE)
                ro = gofs + span * nfull * elem
                while rem > 0:
                    sub = 4 if rem >= 4 else 2
                    rhs = gt[:, ro:ro + sub * elem].rearrange(
                        "p (two f) -> p two f", two=2)
                    nc.tensor.matmul(agg[:, 0:(sub // 2) * elem], lhsT=ip3,
                                     rhs=rhs, start=False,
                                     stop=(rem - sub == 0), perf_mode=DRMODE)
                    ro += sub * elem
                    rem -= sub
                tmp = sb.tile([P, 256], F16, tag="tmp", name=f"tmp{b}")
                nc.scalar.activation(out=tmp[:], in_=agg[:, 256:512],
                                     func=mybir.ActivationFunctionType.Copy,
                                     bias=0.0, scale=1.0)
                if is_b:
                    hsum = sb.tile([P, HID], F16, tag="hsum", name=f"hsum{b}")
                    nc.vector.tensor_tensor(out=hsum[:], in0=agg[:, 0:HID],
                                            in1=tmp[:],
                                            op=mybir.AluOpType.add)
                    h = hp.tile([P, HID], F16, tag=f"h{b}", name=f"h{b}")
                    nc.scalar.activation(out=h[:], in_=hsum[:],
                                         func=mybir.ActivationFunctionType.Relu,
                                         bias=0.0, scale=1.0 / SCALE_B)
                    hs.append(h)
                else:
                    # C: agg = [q0|q1|q2|q3] (128 each); o = (q0+q2)+(q1+q3)
                    s1 = sb.tile([P, 256], F16, tag="s1", name=f"s1{b}")
                    nc.vector.tensor_tensor(out=s1[:], in0=agg[:, 0:256],
                                            in1=tmp[:],
                                            op=mybir.AluOpType.add)
                    o = sb.tile([P, FOUT], F16, tag="o", name=f"o{b}")
                    nc.vector.tensor_tensor(out=o[:], in0=s1[:, 0:FOUT],
                                            in1=s1[:, FOUT:2 * FOUT],
                                            op=mybir.AluOpType.add)
                    nc.scalar.dma_start(t_o[b * P:(b + 1) * P, :], o[:])
                off += nch

            # phase 2 (B only): transpose h; batched Z^T = W2^T h^T over
            # groups of 4 blocks (512-wide zp matmuls), write out.
            if is_b:
                GB = 4

                def _epi2(g0, nb):
                    wdt = nb * P
                    zp = ps3.tile([FOUT, GB * P], F32, tag="zp",
                                  name=f"zp{g0}")
                    for k in range(2):
                        nc.tensor.matmul(zp[:, 0:wdt],
                                         lhsT=w2t[:, k * FOUT:(k + 1) * FOUT],
                                         rhs=hts_cur[k][:, 0:wdt],
                                         start=(k == 0), stop=(k == 1))
                    z = sb.tile([FOUT, GB * P], F16, tag="z", name=f"z{g0}")
                    nc.vector.tensor_copy(z[:, 0:wdt], zp[:, 0:wdt])
                    nc.scalar.dma_start(t_o[:, g0 * P:(g0 + nb) * P],
                                        z[:, 0:wdt])

                hts_cur = None
                prev_grp = None
                for b in range(NBLK):
                    gi = b % GB
                    if gi == 0:
                        if prev_grp is not None:
                            _epi2(*prev_grp)
                            prev_grp = None
                        hts_cur = [
                            sb.tile([P, GB * P], F16, tag=f"hts{k}",
                                    name=f"hts{k}_{b}")
                            for k in range(2)]
                    for k in range(2):
                        htp = ps2.tile([P, P], F16, tag="htp",
                                       name=f"htp{k}_{b}")
                        nc.tensor.transpose(htp[:],
                                            hs[b][:, k * P:(k + 1) * P],
                                            i16[:])
                        nc.vector.tensor_copy(
                            hts_cur[k][:, gi * P:(gi + 1) * P], htp[:])
                    if gi == GB - 1:
                        prev_grp = (b - GB + 1, GB)
                if prev_grp is not None:
                    _epi2(*prev_grp)
                if NBLK % GB:
                    _epi2(NBLK - NBLK % GB, NBLK % GB)
    nc.compile()
    return nc


_KERNEL_CACHE = {}


def _get_kernels(cp):
    key = tuple(int(x) for x in cp)
    if key not in _KERNEL_CACHE:
        _KERNEL_CACHE[key] = (
            _build_a(),
            _build_agg(cp, True),
            _build_agg(cp, False),
        )
    return _KERNEL_CACHE[key]


def kernel(x, edge_index, W1, b1, W2, b2):
    trace = bool(int(os.environ.get("GCN_TRACE", "0")))
    if trace:
        _ensure_ntff_hook()
    exec_ns = []

    def _run(nc, in_maps):
        res = run_bass_kernel_spmd(nc, in_maps, core_ids=list(range(NCORES)),
                                   trace=trace)
        if trace:
            exec_ns.append(res.exec_time_ns)
        return res.results

    x = np.asarray(x)
    edge_index = np.asarray(edge_index)
    W1 = np.asarray(W1, np.float32)
    b1 = np.asarray(b1, np.float32)
    W2 = np.asarray(W2, np.float32)
    b2 = np.asarray(b2, np.float32)

    pre, cp, coff, ct, dinv = _preprocess(edge_index)
    nc_a, nc_b, nc_c = _get_kernels(cp)

    ident16 = np.eye(P, dtype=np.float16)
    ipair8 = np.concatenate([np.eye(P), np.eye(P)], axis=1).astype(F8NP)

    # ---- launch A: T = x @ W1 (per-core node shard) ----
    w1_f16 = W1.astype(np.float16)
    in_a = []
    for c in range(NCORES):
        xs = np.zeros((NLOC_PAD, FIN), np.float16)
        xs[:NLOC] = x[c * NLOC:(c + 1) * NLOC].astype(np.float16)
        xtb = np.ascontiguousarray(
            xs.reshape(NBLK, P, _KC, P).transpose(0, 3, 2, 1)
            .reshape(NBLK, P, FIN))
        in_a.append({"xtb": xtb, "w1": w1_f16})
    res_a = _run(nc_a, in_a)
    tfull = np.concatenate([res_a[c]["t_out"][:NLOC] for c in range(NCORES)],
                           axis=0)                 # [N, HID] f16

    # ---- launch B: h = relu(agg(T)+b1); Z^T = W2^T h^T ----
    w2_f16 = W2.astype(np.float16)
    b1_any = bool(np.any(b1))
    in_b = []
    for c in range(NCORES):
        nb16 = (pre[c]["normmat"] * SCALE_B).astype(np.float16)
        gex = tfull[pre[c]["srcmat"]]              # [128, ct, 256] f16
        gex = gex * nb16[:, :, None]
        if b1_any:
            mask = pre[c]["normmat"][:, coff] != 0
            gex[:, coff, :] += np.where(
                mask[:, :, None], (b1 * SCALE_B).astype(np.float16)[None, None],
                np.float16(0))
        gexp = gex.reshape(P, ct * HID).astype(F8NP)
        in_b.append({"gexp": gexp, "ipair": ipair8, "i16": ident16,
                     "w2": w2_f16})
    res_b = _run(nc_b, in_b)
    zslots = [res_b[c]["zt_out"].T for c in range(NCORES)]   # [6272, 128] f16
    zfull = np.concatenate(
        [zslots[c][pre[c]["perm"]] for c in range(NCORES)], axis=0)

    # ---- launch C: out = agg(Z)/64 + b2 ----
    in_c = []
    for c in range(NCORES):
        nc16 = (pre[c]["normmat"] * SCALE_C).astype(np.float16)
        zex = zfull[pre[c]["srcmat"]]              # [128, ct, 128] f16
        zex = zex * nc16[:, :, None]
        zexp = zex.reshape(P, ct * FOUT).astype(F8NP)
        in_c.append({"gexp": zexp, "ipair": ipair8})
    res_c = _run(nc_c, in_c)
    out = np.concatenate(
        [res_c[c]["ot_out"][pre[c]["perm"]] for c in range(NCORES)], axis=0)
    out = out.astype(np.float32) * (1.0 / SCALE_C) + b2[None, :]

    if trace:
        ns = [int(t) if t else 0 for t in exec_ns]
        print(f"GCN launch exec times (ns): {ns}  total: {sum(ns)}")
        kernel.last_exec_ns = ns
    return np.ascontiguousarray(out.astype(np.float32))


# revision 37
# speedup vs baseline: 1.1448x; 1.1448x over previous
"""Distributed 2-layer GCN on 8 Trainium2 NeuronCores (Bass/Tile).

Strategy (node partition over 8 cores, host-mediated halo exchange):
  Launch A: per-core T = x_shard @ W1               (dense f16 matmul)
  host:     allgather T shards -> T_full
  host:     expand per-edge payload  g[e] = T[src(e)] * norm(e) * 16  (fp8)
            into a degree-sorted, slot-aligned layout: dst node = SBUF
            partition (slot), k-th incident edge = k-th chunk column.
  Launch B: per-core aggregation = PSUM accumulation of payload chunks
            via fp8 DoubleRow matmuls with a constant identity lhsT
            (2 chunks per instruction), then h = relu(agg/16),
            transpose (TensorE) and Z^T = W2^T @ h^T.
  host:     allgather Z shards, expand z[e] = Z[src(e)] * norm(e) * 64 (fp8)
  Launch C: same identity-accumulate aggregation, out = agg (f16);
            host applies /64 and + b2.

No dma_gather / GPSIMD anywhere: the gather indices are known on the host
between launches, so all device traffic is large contiguous DMA.  The
one-hot scatter matrices of the old design are gone too - the slot-aligned
layout makes the aggregation a pure chunk sum, which the identity matmul
performs in PSUM at 2 chunks/instruction (fp8 DoubleRow).
All normalization (D^-1/2 (A+I) D^-1/2) is folded into the payload on the
host at f32/f16 precision with a single fp8 quantization per layer.
b1/b2: b1 is added into the self-loop payload rows (exact when b1=0), b2 is
added on the host after the final gather.
"""

import os
import sys
import types

import ml_dtypes
import numpy as np

import concourse.bass as bass
import concourse.bacc as bacc
import concourse.tile as tile
from concourse import mybir
from concourse.bass_utils import run_bass_kernel_spmd

NCORES = 8
N = 50000
FIN = 768
HID = 256
FOUT = 128
NLOC = N // NCORES            # 6250 nodes per core
NBLK = 49                     # dst blocks per core (49*128 = 6272 slots)
P = 128
NLOC_PAD = NBLK * P

SCALE_B = 16.0                # payload scale for layer-1 messages (fp8 range)
SCALE_C = 64.0                # payload scale for layer-2 messages

F16 = mybir.dt.float16
F32 = mybir.dt.float32
F8 = mybir.dt.float8e4
DRMODE = mybir.MatmulPerfMode.DoubleRow
F8NP = ml_dtypes.float8_e4m3fn

_KC = FIN // P  # 6


def _ensure_ntff_hook():
    """Provide antenv.axon_hooks if the image lacks it, so trace=True works."""
    try:
        import antenv.axon_hooks  # noqa: F401
        return
    except ImportError:
        pass
    import antenv
    mod = types.ModuleType("antenv.axon_hooks")
    mod._hook = None

    def set_axon_ntff_profile_hook(hook):
        mod._hook = hook

    def get_axon_ntff_profile_hook():
        return mod._hook

    mod.set_axon_ntff_profile_hook = set_axon_ntff_profile_hook
    mod.get_axon_ntff_profile_hook = get_axon_ntff_profile_hook
    sys.modules["antenv.axon_hooks"] = mod
    antenv.axon_hooks = mod
    try:
        from trn_agent_boot.trn_boot import _ntff_profile_via_ctypes
        hook = _ntff_profile_via_ctypes("/opt/axon/libaxon_pjrt.so")
        if hook is not None:
            mod._hook = hook
    except Exception:
        pass


def _preprocess(edge_index):
    """Degree-sorted node->(block, slot) assignment per core plus the
    (slot, chunk) placement of every edge (self-loops at chunk 0)."""
    src = edge_index[0].astype(np.int64)
    dst = edge_index[1].astype(np.int64)
    deg = np.bincount(dst, minlength=N).astype(np.float64) + 1.0  # incl self
    dinv = 1.0 / np.sqrt(deg)

    perms = []
    prof = np.zeros(NBLK, np.int64)
    for c in range(NCORES):
        lo = c * NLOC
        dloc = deg[lo:lo + NLOC].astype(np.int64)
        order = np.argsort(-dloc, kind="stable")
        perm_slots = np.empty(NLOC, np.int64)
        perm_slots[order] = np.arange(NLOC)     # node -> b*128 + slot
        dpad = np.zeros(NLOC_PAD, np.int64)
        dpad[:NLOC] = dloc[order]
        cpb = dpad.reshape(NBLK, P).max(axis=1)
        cpb = ((cpb + 1) // 2) * 2              # even for DoubleRow pairing
        prof = np.maximum(prof, cpb)
        perms.append(perm_slots)

    cp = np.maximum(prof, 8)                     # aligned chunk profile
                                                 # (>=8 so C's oct loop runs)
    coff = np.concatenate([[0], np.cumsum(cp)])[:-1].astype(np.int64)
    ct = int(cp.sum())

    pre = []
    for c in range(NCORES):
        lo = c * NLOC
        perm_slots = perms[c]
        sel = (dst >= lo) & (dst < lo + NLOC)
        s_c = src[sel]
        d_glob = dst[sel]
        d_c = d_glob - lo
        n_c = (dinv[s_c] * dinv[d_glob]).astype(np.float32)
        o = np.argsort(d_c, kind="stable")
        s_c, d_c, n_c = s_c[o], d_c[o], n_c[o]
        cnt = np.bincount(d_c, minlength=NLOC)
        starts = np.zeros(NLOC, np.int64)
        starts[1:] = np.cumsum(cnt)[:-1]
        kpos = np.arange(len(d_c)) - starts[d_c] + 1   # 1.. (0 = self)
        pos = perm_slots[d_c]
        blk, slot = pos // P, pos % P
        col = coff[blk] + kpos

        srcmat = np.zeros((P, ct), np.int64)
        normmat = np.zeros((P, ct), np.float32)
        srcmat[slot, col] = s_c
        normmat[slot, col] = n_c
        # self loops at chunk 0 of each block
        nodes = np.arange(NLOC)
        posn = perm_slots[nodes]
        blkn, slotn = posn // P, posn % P
        srcmat[slotn, coff[blkn]] = lo + nodes
        normmat[slotn, coff[blkn]] = (dinv[lo + nodes] ** 2).astype(np.float32)
        pre.append({"perm": posn, "srcmat": srcmat, "normmat": normmat})
    return pre, cp, coff, ct, dinv


def _build_a():
    nc = bacc.Bacc("TRN2", target_bir_lowering=False, debug=False,
                   num_devices=NCORES)
    # host-swizzled so each block loads as one contiguous-per-partition DMA:
    # xtb[b, p, k*128+n] = x[b*128+n, k*128+p]
    t_xt = nc.dram_tensor("xtb", [NBLK, P, FIN], F16, kind="ExternalInput")
    t_w1 = nc.dram_tensor("w1", [FIN, HID], F16, kind="ExternalInput")
    t_out = nc.dram_tensor("t_out", [NLOC_PAD, HID], F16, kind="ExternalOutput")
    with tile.TileContext(nc) as tc:
        with (
            tc.tile_pool(name="const", bufs=1) as cs,
            tc.tile_pool(name="sb", bufs=8) as sb,
            tc.tile_pool(name="tp", bufs=1) as tp,
            tc.tile_pool(name="ps", bufs=4, space="PSUM") as ps,
        ):
            w1t = cs.tile([P, _KC * HID], F16)
            for k in range(_KC):
                nc.sync.dma_start(w1t[:, k * HID:(k + 1) * HID],
                                  t_w1[k * P:(k + 1) * P, :])

            def _epilogue_a(b, pt):
                ts = tp.tile([P, HID], F16, tag=f"ts{b}", name=f"ts{b}")
                nc.vector.tensor_copy(ts[:], pt[:])
                nc.scalar.dma_start(t_out[b * P:(b + 1) * P, :], ts[:])

            prev = None
            for b in range(NBLK):
                xts = sb.tile([P, FIN], F16, tag="xt")
                nc.sync.dma_start(xts[:], t_xt[b])
                pt = ps.tile([P, HID], F32, tag="pt")
                for k in range(_KC):
                    nc.tensor.matmul(pt[:], lhsT=xts[:, k * P:(k + 1) * P],
                                     rhs=w1t[:, k * HID:(k + 1) * HID],
                                     start=(k == 0), stop=(k == _KC - 1))
                if prev is not None:
                    _epilogue_a(*prev)
                prev = (b, pt)
            _epilogue_a(*prev)
    nc.compile()
    return nc


def _build_agg(cp, is_b):
    """Aggregation launch: identity-accumulate over slot-aligned payload.
    B (elem=HID): h = relu(agg/16), transpose, Z^T = W2^T h^T.
    C (elem=FOUT): out = agg (f16)."""
    ct = int(np.sum(cp))
    cpmax = int(np.max(cp))
    elem = HID if is_b else FOUT
    nc = bacc.Bacc("TRN2", target_bir_lowering=False, debug=False,
                   num_devices=NCORES)
    t_g = nc.dram_tensor("gexp", [P, ct * elem], F8, kind="ExternalInput")
    t_ip = nc.dram_tensor("ipair", [P, 2 * P], F8, kind="ExternalInput")
    if is_b:
        t_i16 = nc.dram_tensor("i16", [P, P], F16, kind="ExternalInput")
        t_w2 = nc.dram_tensor("w2", [HID, FOUT], F16, kind="ExternalInput")
        t_o = nc.dram_tensor("zt_out", [FOUT, NLOC_PAD], F16,
                             kind="ExternalOutput")
    else:
        t_o = nc.dram_tensor("ot_out", [NLOC_PAD, FOUT], F16,
                             kind="ExternalOutput")

    with tile.TileContext(nc) as tc:
        with (
            tc.tile_pool(name="const", bufs=1) as cs,
            tc.tile_pool(name="gp", bufs=10 if is_b else 4) as gp,
            tc.tile_pool(name="hp", bufs=1) as hp,
            tc.tile_pool(name="sb", bufs=8) as sb,
            tc.tile_pool(name="ps", bufs=3 if is_b else 8, space="PSUM") as ps,
            tc.tile_pool(name="ps2", bufs=2, space="PSUM") as ps2,
            tc.tile_pool(name="ps3", bufs=3, space="PSUM") as ps3,
        ):
            ip = cs.tile([P, 2 * P], F8)
            nc.sync.dma_start(ip[:], t_ip[:, :])
            ip3 = ip[:].rearrange("p (two f) -> p two f", two=2)
            if is_b:
                i16 = cs.tile([P, P], F16)
                nc.sync.dma_start(i16[:], t_i16[:, :])
                w2t = cs.tile([P, 2 * FOUT], F16)
                for k in range(2):
                    nc.sync.dma_start(w2t[:, k * FOUT:(k + 1) * FOUT],
                                      t_w2[k * P:(k + 1) * P, :])

            # phase 1: payload DMA + identity-DR aggregation, chain after
            # chain. Each quad-DR matmul contracts 4 chunks into two
            # side-by-side half-aggregates; DVE adds the halves. h lands in
            # a distinct SBUF tile per block (no ring WAR), so TensorE
            # streams without cross-engine stalls.
            hs = []
            # C: group 4 blocks per payload DMA (fewer, larger transfers);
            # B: per-block DMA (bus-bound anyway, SBUF is tighter there).
            grp = 1 if is_b else 4
            goffs = {}
            gtiles = {}
            off = 0
            for b in range(NBLK):
                nch = int(cp[b])
                if b % grp == 0:
                    gn = sum(int(cp[i]) for i in range(b, min(b + grp, NBLK)))
                    gt = gp.tile([P, grp * cpmax * elem], F8, tag="g")
                    nc.sync.dma_start(gt[:, 0:gn * elem],
                                      t_g[:, off * elem:(off + gn) * elem])
                    gtiles[b] = gt
                    goffs[b] = 0
                else:
                    gtiles[b] = gtiles[b - 1]
                    goffs[b] = goffs[b - 1] + int(cp[b - 1])
                gt = gtiles[b]
                gofs = goffs[b] * elem
                # per-instruction chunk span: B quads (free 2x512), C octs
                # (free 2x512 = 8 chunks of 128) -> PSUM [128, W] partials
                span = 4 if is_b else 8
                agg = ps.tile([P, 512], F32, tag="agg")
                nfull, rem = nch // span, nch % span
                for j in range(nfull):
                    rhs = gt[:, gofs + span * j * elem:
                             gofs + span * (j + 1) * elem].rearrange(
                        "p (two f) -> p two f", two=2)
                    nc.tensor.matmul(agg[:], lhsT=ip3, rhs=rhs,
                                     start=(j == 0),
                                     stop=(rem == 0 and j == nfull - 1),
                                     perf_mode=DRMODE)
                ro = gofs + span * nfull * elem
                while rem > 0:
                    sub = 4 if rem >= 4 else 2
                    rhs = gt[:, ro:ro + sub * elem].rearrange(
                        "p (two f) -> p two f", two=2)
                    nc.tensor.matmul(agg[:, 0:(sub // 2) * elem], lhsT=ip3,
                                     rhs=rhs, start=False,
                                     stop=(rem - sub == 0), perf_mode=DRMODE)
                    ro += sub * elem
                    rem -= sub
                tmp = sb.tile([P, 256], F16, tag="tmp", name=f"tmp{b}")
                if is_b:
                    nc.scalar.activation(out=tmp[:], in_=agg[:, 256:512],
                                         func=mybir.ActivationFunctionType.Copy,
                                         bias=0.0, scale=1.0)
                else:
                    nc.vector.tensor_copy(tmp[:], agg[:, 256:512])
                if is_b:
                    hsum = sb.tile([P, HID], F16, tag="hsum", name=f"hsum{b}")
                    nc.vector.tensor_tensor(out=hsum[:], in0=agg[:, 0:HID],
                                            in1=tmp[:],
                                            op=mybir.AluOpType.add)
                    h = hp.tile([P, HID], F16, tag=f"h{b}", name=f"h{b}")
                    nc.scalar.activation(out=h[:], in_=hsum[:],
                                         func=mybir.ActivationFunctionType.Relu,
                                         bias=0.0, scale=1.0 / SCALE_B)
                    hs.append(h)
                else:
                    # C: agg = [q0|q1|q2|q3] (128 each); o = (q0+q2)+(q1+q3)
                    s1 = sb.tile([P, 256], F16, tag="s1", name=f"s1{b}")
                    nc.vector.tensor_tensor(out=s1[:], in0=agg[:, 0:256],
                                            in1=tmp[:],
                                            op=mybir.AluOpType.add)
                    o = sb.tile([P, FOUT], F16, tag="o", name=f"o{b}")
                    nc.vector.tensor_tensor(out=o[:], in0=s1[:, 0:FOUT],
                                            in1=s1[:, FOUT:2 * FOUT],
                                            op=mybir.AluOpType.add)
                    nc.scalar.dma_start(t_o[b * P:(b + 1) * P, :], o[:])
                off += nch

            # phase 2 (B only): transpose h; batched Z^T = W2^T h^T over
            # groups of 4 blocks (512-wide zp matmuls), write out.
            if is_b:
                GB = 4

                def _epi2(g0, nb):
                    wdt = nb * P
                    zp = ps3.tile([FOUT, GB * P], F32, tag="zp",
                                  name=f"zp{g0}")
                    for k in range(2):
                        nc.tensor.matmul(zp[:, 0:wdt],
                                         lhsT=w2t[:, k * FOUT:(k + 1) * FOUT],
                                         rhs=hts_cur[k][:, 0:wdt],
                                         start=(k == 0), stop=(k == 1))
                    z = sb.tile([FOUT, GB * P], F16, tag="z", name=f"z{g0}")
                    nc.vector.tensor_copy(z[:, 0:wdt], zp[:, 0:wdt])
                    nc.scalar.dma_start(t_o[:, g0 * P:(g0 + nb) * P],
                                        z[:, 0:wdt])

                hts_cur = None
                prev_grp = None
                for b in range(NBLK):
                    gi = b % GB
                    if gi == 0:
                        if prev_grp is not None:
                            _epi2(*prev_grp)
                            prev_grp = None
                        hts_cur = [
                            sb.tile([P, GB * P], F16, tag=f"hts{k}",
                                    name=f"hts{k}_{b}")
                            for k in range(2)]
                    for k in range(2):
                        htp = ps2.tile([P, P], F16, tag="htp",
                                       name=f"htp{k}_{b}")
                        nc.tensor.transpose(htp[:],
                                            hs[b][:, k * P:(k + 1) * P],
                                            i16[:])
                        nc.vector.tensor_copy(
                            hts_cur[k][:, gi * P:(gi + 1) * P], htp[:])
                    if gi == GB - 1:
                        prev_grp = (b - GB + 1, GB)
                if prev_grp is not None:
                    _epi2(*prev_grp)
                if NBLK % GB:
                    _epi2(NBLK - NBLK % GB, NBLK % GB)
    nc.compile()
    return nc


_KERNEL_CACHE = {}


def _get_kernels(cp):
    key = tuple(int(x) for x in cp)
    if key not in _KERNEL_CACHE:
        _KERNEL_CACHE[key] = (
            _build_a(),
            _build_agg(cp, True),
            _build_agg(cp, False),
        )
    return _KERNEL_CACHE[key]


def kernel(x, edge_index, W1, b1, W2, b2):
    trace = bool(int(os.environ.get("GCN_TRACE", "0")))
    if trace:
        _ensure_ntff_hook()
    exec_ns = []

    def _run(nc, in_maps):
        res = run_bass_kernel_spmd(nc, in_maps, core_ids=list(range(NCORES)),
                                   trace=trace)
        if trace:
            exec_ns.append(res.exec_time_ns)
        return res.results

    x = np.asarray(x)
    edge_index = np.asarray(edge_index)
    W1 = np.asarray(W1, np.float32)
    b1 = np.asarray(b1, np.float32)
    W2 = np.asarray(W2, np.float32)
    b2 = np.asarray(b2, np.float32)

    pre, cp, coff, ct, dinv = _preprocess(edge_index)
    nc_a, nc_b, nc_c = _get_kernels(cp)

    ident16 = np.eye(P, dtype=np.float16)
    ipair8 = np.concatenate([np.eye(P), np.eye(P)], axis=1).astype(F8NP)

    # ---- launch A: T = x @ W1 (per-core node shard) ----
    w1_f16 = W1.astype(np.float16)
    in_a = []
    for c in range(NCORES):
        xs = np.zeros((NLOC_PAD, FIN), np.float16)
        xs[:NLOC] = x[c * NLOC:(c + 1) * NLOC].astype(np.float16)
        xtb = np.ascontiguousarray(
            xs.reshape(NBLK, P, _KC, P).transpose(0, 3, 2, 1)
            .reshape(NBLK, P, FIN))
        in_a.append({"xtb": xtb, "w1": w1_f16})
    res_a = _run(nc_a, in_a)
    tfull = np.concatenate([res_a[c]["t_out"][:NLOC] for c in range(NCORES)],
                           axis=0)                 # [N, HID] f16

    # ---- launch B: h = relu(agg(T)+b1); Z^T = W2^T h^T ----
    w2_f16 = W2.astype(np.float16)
    b1_any = bool(np.any(b1))
    in_b = []
    for c in range(NCORES):
        nb16 = (pre[c]["normmat"] * SCALE_B).astype(np.float16)
        gex = tfull[pre[c]["srcmat"]]              # [128, ct, 256] f16
        gex = gex * nb16[:, :, None]
        if b1_any:
            mask = pre[c]["normmat"][:, coff] != 0
            gex[:, coff, :] += np.where(
                mask[:, :, None], (b1 * SCALE_B).astype(np.float16)[None, None],
                np.float16(0))
        gexp = gex.reshape(P, ct * HID).astype(F8NP)
        in_b.append({"gexp": gexp, "ipair": ipair8, "i16": ident16,
                     "w2": w2_f16})
    res_b = _run(nc_b, in_b)
    zslots = [res_b[c]["zt_out"].T for c in range(NCORES)]   # [6272, 128] f16
    zfull = np.concatenate(
        [zslots[c][pre[c]["perm"]] for c in range(NCORES)], axis=0)

    # ---- launch C: out = agg(Z)/64 + b2 ----
    in_c = []
    for c in range(NCORES):
        nc16 = (pre[c]["normmat"] * SCALE_C).astype(np.float16)
        zex = zfull[pre[c]["srcmat"]]              # [128, ct, 128] f16
        zex = zex * nc16[:, :, None]
        zexp = zex.reshape(P, ct * FOUT).astype(F8NP)
        in_c.append({"gexp": zexp, "ipair": ipair8})
    res_c = _run(nc_c, in_c)
    out = np.concatenate(
        [res_c[c]["ot_out"][pre[c]["perm"]] for c in range(NCORES)], axis=0)
    out = out.astype(np.float32) * (1.0 / SCALE_C) + b2[None, :]

    if trace:
        ns = [int(t) if t else 0 for t in exec_ns]
        print(f"GCN launch exec times (ns): {ns}  total: {sum(ns)}")
        kernel.last_exec_ns = ns
    return np.ascontiguousarray(out.astype(np.float32))
